# revision 1
# baseline (speedup 1.0000x reference)
"""Fused rotary QK-projection + normalized dot-product attention softmax.

Computes softmax((q_hat @ k_hat^T) / 64) for q,k = L2-normalized rotary
projections of x, sharded over 8 NeuronCores as (batch x head-pair):
core c -> batch c//4, heads (2*(c%4), 2*(c%4)+1). No cross-core comms.

Self-contained: hardcodes shapes b=2, n=2048, dim=512, h=8, d=64.
"""

import numpy as np
import ml_dtypes

B = 2
N = 2048
C = 512           # model dim (contraction for projection)
H = 8             # heads
D = 64            # head dim
HPC = 2           # heads per core
NCORES = 8
KC = C // 128     # 4 contraction chunks of 128
NJ = N // 512     # 4 chain chunks of 512
NT = N // 128     # 16 q row-tiles

_CACHE = {}


def _setup_act_tables():
    """Point walrus at an act_info.json tweaked so Ln and Exp both resolve
    to natural_log_exp_and_others (one shared ACT table set -> no ~1.3us
    table reloads between Ln and Exp activations). Set order/indices are
    kept identical; only the per-func set choice is steered."""
    import os
    import json
    import tempfile
    from pathlib import Path

    if os.environ.get("BASS_ACT_ROOT_JSON_PATH"):
        return
    from neuronxcc.driver.Job import Job

    src_dir = Path(Job.getPackageDir()) / "pwp" / "pwp_bin_trainium"
    src_json = src_dir / "act_info.json"
    if not src_json.exists():
        return
    info = json.loads(src_json.read_text())
    sets = info.get("act_func_sets", [])
    names = [s.get("name") for s in sets]
    if "natural_log_exp_and_others" not in names:
        return
    for s in sets:
        if s.get("name") == "exp_and_others":
            s.get("act", {}).pop("exp", None)
        elif s.get("name") == "natural_log":
            s.get("act", {}).pop("ln", None)
    dst_dir = Path(tempfile.mkdtemp(prefix="pwp_act_"))
    for f in src_dir.iterdir():
        if f.name != "act_info.json":
            (dst_dir / f.name).symlink_to(f)
    (dst_dir / "act_info.json").write_text(json.dumps(info))
    os.environ["BASS_ACT_ROOT_JSON_PATH"] = str(dst_dir / "act_info.json")


def _build_nc():
    import concourse.mybir as mybir
    import concourse.tile as tile
    from concourse import bacc

    _setup_act_tables()

    dt = mybir.dt
    f32, bf16 = dt.float32, dt.bfloat16
    AF = mybir.ActivationFunctionType

    nc = bacc.Bacc(None)
    # partition-major host layouts -> contiguous per-partition DMA segments
    xT = nc.dram_tensor("xT", [128, NJ, KC, 512], bf16, kind="ExternalInput")
    wq = nc.dram_tensor("wq", [128, HPC, KC, 128], bf16, kind="ExternalInput")
    wr = nc.dram_tensor("wr", [128, HPC, KC, 128], bf16, kind="ExternalInput")
    cosr = nc.dram_tensor("cosr", [128, N], bf16, kind="ExternalInput")
    sinr = nc.dram_tensor("sinr", [128, N], bf16, kind="ExternalInput")
    maskt = nc.dram_tensor("maskt", [128, 2], bf16, kind="ExternalInput")
    out = nc.dram_tensor("out", [HPC, N, N], f32, kind="ExternalOutput")
    # bounce rows: per-head 1/|q_n| (partition-gathered) and 1/|k_n|
    # (partition-broadcast)
    rqd = nc.dram_tensor("rqd", [HPC, N], bf16)
    rkd = nc.dram_tensor("rkd", [HPC, N], bf16)

    with tile.TileContext(nc) as tc:
        with (
            tc.tile_pool(name="singles", bufs=1) as singles,
            tc.tile_pool(name="chain", bufs=2) as chain_pool,
            tc.tile_pool(name="persist", bufs=2) as persist,
            tc.tile_pool(name="exp", bufs=4) as exp_pool,
            tc.tile_pool(name="outp", bufs=6) as out_pool,
            tc.tile_pool(name="small", bufs=8) as small,
            tc.tile_pool(name="pchain", bufs=2, space="PSUM") as pchain,
            tc.tile_pool(name="psc", bufs=2, space="PSUM") as psc,
        ):
            wqt = singles.tile([128, HPC, KC, 128], bf16)
            nc.sync.dma_start(out=wqt[:], in_=wq[:])
            wrt = singles.tile([128, HPC, KC, 128], bf16)
            nc.sync.dma_start(out=wrt[:], in_=wr[:])
            mask = singles.tile([128, 2], bf16)
            nc.sync.dma_start(out=mask[:], in_=maskt[:])
            cost = singles.tile([128, N], bf16)
            sint = singles.tile([128, N], bf16)
            xt = singles.tile([128, NJ, KC, 512], bf16)
            for j in range(NJ):
                nc.sync.dma_start(out=xt[:, j, :, :], in_=xT[:, j, :, :])
            for j in range(NJ):
                js = slice(j * 512, (j + 1) * 512)
                nc.sync.dma_start(out=cost[:, js], in_=cosr[:, js])
                nc.sync.dma_start(out=sint[:, js], in_=sinr[:, js])

            # per-head persistent tiles (indexed by head)
            qr_t, kt_t, nsq_t, rqs_t = {}, {}, {}, {}

            def chain_start(t):
                # [e, n] layout: partitions = 64 q-dims then 64 k-dims.
                qr_t[t] = persist.tile([128, N], bf16, tag="qr", name=f"qr{t}")
                kt_t[t] = persist.tile([64, N], bf16, tag="kt", name=f"kt{t}")
                nsq_t[t] = persist.tile([2, N], f32, tag="nsq", name=f"nsq{t}")

            def chain_chunk(t, j):
                qr, nsq_sb = qr_t[t], nsq_t[t]
                js = slice(j * 512, (j + 1) * 512)
                qk_ps = pchain.tile([128, 512], f32, tag="pq")
                rot_ps = pchain.tile([128, 512], f32, tag="pr")
                for k in range(KC):
                    nc.tensor.matmul(
                        qk_ps[:], lhsT=wqt[:, t, k, :], rhs=xt[:, j, k, :],
                        start=(k == 0), stop=(k == KC - 1),
                    )
                for k in range(KC):
                    nc.tensor.matmul(
                        rot_ps[:], lhsT=wrt[:, t, k, :], rhs=xt[:, j, k, :],
                        start=(k == 0), stop=(k == KC - 1),
                    )
                # rotary: qr = qk*cos + rot(qk)*sin
                t1 = chain_pool.tile([128, 512], bf16, tag="t1")
                nc.vector.tensor_mul(t1[:], qk_ps[:], cost[:, js])
                t2 = chain_pool.tile([128, 512], bf16, tag="t2")
                nc.vector.tensor_mul(t2[:], rot_ps[:], sint[:, js])
                nc.vector.tensor_add(qr[:, js], t1[:], t2[:])
                # squared L2 norms along d (partition dim) via mask matmul
                sq = chain_pool.tile([128, 512], bf16, tag="sq")
                nc.vector.tensor_mul(sq[:], qr[:, js], qr[:, js])
                nsq_ps = pchain.tile([2, 512], f32, tag="pq")
                nc.tensor.matmul(
                    nsq_ps[:], lhsT=mask[:], rhs=sq[:], start=True, stop=True
                )
                nc.vector.tensor_copy(nsq_sb[:, js], nsq_ps[:])

            def chain_norm(t):
                # rinorm = nsq^-0.5 = exp(-0.5*ln(nsq)), batched per head
                # (2 ACT table switches); the Exp side is chunked so the
                # downstream k-normalize pipeline starts per chunk.
                nsq_sb = nsq_t[t]
                rin = chain_pool.tile([2, N], bf16, tag="rin", name=f"rin{t}")
                lnn = chain_pool.tile([2, N], f32, tag="lnn", name=f"lnn{t}")
                nc.scalar.activation(out=lnn[:], in_=nsq_sb[:], func=AF.Ln)
                nc.scalar.activation(out=rin[:], in_=lnn[:], func=AF.Exp, scale=-0.5)
                nc.sync.dma_start(out=rqd[t, :], in_=rin[0:1, :])
                nc.sync.dma_start(out=rkd[t, :], in_=rin[1:2, :])

            def chain_knorm(t, j):
                # kt = (k-half of qr, shifted to base partition 0) * 1/|k|
                qr, kt = qr_t[t], kt_t[t]
                js = slice(j * 512, (j + 1) * 512)
                ks = chain_pool.tile([64, 512], bf16, tag="ks")
                nc.sync.dma_start(out=ks[:], in_=qr[64:128, js])
                bc = chain_pool.tile([64, 512], bf16, tag="bc")
                nc.sync.dma_start(out=bc[:], in_=rkd[t:t + 1, js].to_broadcast([64, 512]))
                nc.vector.tensor_mul(kt[:, js], ks[:], bc[:])

            def chain_finish(t):
                # per-q-tile exp scales: gather 1/|q| across partitions, /64
                rqt = small.tile([128, NT], bf16, tag="rqt")
                nc.sync.dma_start(
                    out=rqt[:], in_=rqd[t].rearrange("(i p) -> p i", p=128)
                )
                rqs = small.tile([128, NT], f32, tag="rqs", name=f"rqs{t}")
                nc.vector.tensor_scalar_mul(rqs[:], rqt[:], 1.0 / D)
                rqs_t[t] = rqs

            def scores_tile(t, i):
                qr, kt, rqs = qr_t[t], kt_t[t], rqs_t[t]
                isl = slice(i * 128, (i + 1) * 128)
                et = exp_pool.tile([128, N], f32, tag="et")
                sums = small.tile([128, 2], f32, tag="sums")
                for hlf in range(2):
                    hs = slice(hlf * 1024, (hlf + 1) * 1024)
                    sc_ps = psc.tile([128, 1024], f32, tag="sc")
                    for j2 in range(2):
                        js2 = slice(hlf * 1024 + j2 * 512, hlf * 1024 + (j2 + 1) * 512)
                        nc.tensor.matmul(
                            sc_ps[:, j2 * 512:(j2 + 1) * 512],
                            lhsT=qr[0:64, isl], rhs=kt[:, js2],
                            start=True, stop=True,
                        )
                    nc.scalar.activation(
                        out=et[:, hs], in_=sc_ps[:], func=AF.Exp,
                        scale=rqs[:, i:i + 1], accum_out=sums[:, hlf:hlf + 1],
                    )
                ssum = small.tile([128, 1], f32, tag="ssum")
                nc.vector.tensor_tensor(
                    out=ssum[:], in0=sums[:, 0:1], in1=sums[:, 1:2],
                    op=mybir.AluOpType.add,
                )
                rs = small.tile([128, 1], f32, tag="rs")
                nc.vector.reciprocal(out=rs[:], in_=ssum[:])
                ot = out_pool.tile([128, N], f32, tag="ot")
                nc.vector.tensor_scalar_mul(ot[:], et[:], rs[:])
                nc.sync.dma_start(out=out[t, isl, :], in_=ot[:])

            # software pipeline: head-1 chain interleaved into head-0 scores
            chain_start(0)
            for j in range(NJ):
                chain_chunk(0, j)
            chain_norm(0)
            chain_finish(0)
            for j in range(NJ):
                chain_knorm(0, j)
            chain_start(1)
            for i in range(NT):
                scores_tile(0, i)
                if i in (1, 3, 5, 7):
                    chain_chunk(1, (i - 1) // 2)
                elif i == 9:
                    chain_norm(1)
                    chain_finish(1)
                    for j in range(NJ):
                        chain_knorm(1, j)
            for i in range(NT):
                scores_tile(1, i)

    nc.compile()
    return nc


def _get_nc():
    if "nc" not in _CACHE:
        _CACHE["nc"] = _build_nc()
    return _CACHE["nc"]


def _prep_inputs(x, rotary_cos, rotary_sin, W_qk):
    bf16 = ml_dtypes.bfloat16
    x = np.asarray(x, dtype=np.float32)
    cos = np.asarray(rotary_cos, dtype=np.float32)
    sin = np.asarray(rotary_sin, dtype=np.float32)
    W = np.asarray(W_qk, dtype=np.float32)

    cosr = np.concatenate([cos.T, cos.T], axis=0).astype(bf16)  # [128, N]
    sinr = np.concatenate([sin.T, sin.T], axis=0).astype(bf16)
    maskt = np.zeros((128, 2), dtype=bf16)
    maskt[0:64, 0] = 1.0
    maskt[64:128, 1] = 1.0

    # per-head weight lhsT chunks (and rotate_half-permuted variant),
    # stored partition-major: [p, head, kc, m]
    wq_h = np.empty((H, KC, 128, 128), dtype=np.float32)
    wr_h = np.empty((H, KC, 128, 128), dtype=np.float32)
    for h in range(H):
        wcat = np.concatenate(
            [W[h * D:(h + 1) * D], W[C + h * D:C + (h + 1) * D]], axis=0
        )  # [128, 512]
        wrot = np.empty_like(wcat)
        wrot[0:32] = -wcat[32:64]
        wrot[32:64] = wcat[0:32]
        wrot[64:96] = -wcat[96:128]
        wrot[96:128] = wcat[64:96]
        wq_h[h] = wcat.T.reshape(KC, 128, 128)
        wr_h[h] = wrot.T.reshape(KC, 128, 128)

    # xT partition-major chunked: [p, j, kc, nn]
    xTb = []
    for b in range(B):
        xT = x[b].T  # [C, N]
        xTb.append(np.ascontiguousarray(
            xT.reshape(KC, 128, NJ, 512).transpose(1, 2, 0, 3)
        ).astype(bf16))

    in_maps = []
    for core in range(NCORES):
        b = core // 4
        h0 = (core % 4) * HPC
        wqc = np.ascontiguousarray(
            wq_h[h0:h0 + HPC].transpose(2, 0, 1, 3)
        ).astype(bf16)  # [128, HPC, KC, 128]
        wrc = np.ascontiguousarray(
            wr_h[h0:h0 + HPC].transpose(2, 0, 1, 3)
        ).astype(bf16)
        in_maps.append({
            "xT": xTb[b],
            "wq": wqc,
            "wr": wrc,
            "cosr": cosr,
            "sinr": sinr,
            "maskt": maskt,
        })
    return in_maps


def run(x, rotary_cos, rotary_sin, W_qk, trace=False):
    from concourse.bass_utils import run_bass_kernel_spmd

    nc = _get_nc()
    in_maps = _prep_inputs(x, rotary_cos, rotary_sin, W_qk)
    res = run_bass_kernel_spmd(nc, in_maps, list(range(NCORES)), trace=trace)
    full = np.empty((B, H, N, N), dtype=np.float32)
    for core in range(NCORES):
        b = core // 4
        h0 = (core % 4) * HPC
        for t in range(HPC):
            full[b, h0 + t] = res.results[core]["out"][t]
    return full, res


def kernel(x, rotary_cos, rotary_sin, W_qk):
    full, _ = run(x, rotary_cos, rotary_sin, W_qk, trace=False)
    return full



# revision 2
# speedup vs baseline: 1.2953x; 1.2953x over previous
"""Fused rotary QK-projection + normalized dot-product attention softmax.

Computes softmax((q_hat @ k_hat^T) / 64) for q,k = L2-normalized rotary
projections of x, sharded over 8 NeuronCores as (batch x head-pair):
core c -> batch c//4, heads (2*(c%4), 2*(c%4)+1). No cross-core comms.

Structure: prologue computes both heads' projection chains (rotary,
norms, normalized q/k in SBUF); the score phase then runs pure
[128,2048] matmul->Exp->scale->DMA tiles, bottlenecked by the scalar
engine's exp throughput. Output is written bf16 (halves DMA-out bytes)
and upcast on host.

Self-contained: hardcodes shapes b=2, n=2048, dim=512, h=8, d=64.
"""

import numpy as np
import ml_dtypes

B = 2
N = 2048
C = 512           # model dim (contraction for projection)
H = 8             # heads
D = 64            # head dim
HPC = 2           # heads per core
NCORES = 8
KC = C // 128     # 4 contraction chunks of 128
NJ = N // 512     # 4 chain chunks of 512
NT = N // 128     # 16 q row-tiles

_CACHE = {}


def _setup_act_tables():
    """Point walrus at an act_info.json tweaked so Ln, Exp and Square all
    resolve to natural_log_exp_and_others (one shared ACT table set -> no
    ~2.7us table reloads between activation funcs). Set order/indices are
    kept identical; only the per-func set choice is steered."""
    import os
    import json
    import tempfile
    from pathlib import Path

    if os.environ.get("BASS_ACT_ROOT_JSON_PATH"):
        return
    from neuronxcc.driver.Job import Job

    src_dir = Path(Job.getPackageDir()) / "pwp" / "pwp_bin_trainium"
    src_json = src_dir / "act_info.json"
    if not src_json.exists():
        return
    info = json.loads(src_json.read_text())
    sets = info.get("act_func_sets", [])
    names = [s.get("name") for s in sets]
    if "natural_log_exp_and_others" not in names:
        return
    for s in sets:
        if s.get("name") != "natural_log_exp_and_others":
            s.get("act", {}).pop("exp", None)
            s.get("act", {}).pop("ln", None)
            s.get("act", {}).pop("square", None)
    dst_dir = Path(tempfile.mkdtemp(prefix="pwp_act_"))
    for f in src_dir.iterdir():
        if f.name != "act_info.json":
            (dst_dir / f.name).symlink_to(f)
    (dst_dir / "act_info.json").write_text(json.dumps(info))
    os.environ["BASS_ACT_ROOT_JSON_PATH"] = str(dst_dir / "act_info.json")


def _build_nc():
    import concourse.mybir as mybir
    import concourse.tile as tile
    from concourse import bacc

    _setup_act_tables()

    dt = mybir.dt
    f32, bf16 = dt.float32, dt.bfloat16
    AF = mybir.ActivationFunctionType

    nc = bacc.Bacc(None)
    # partition-major host layouts -> contiguous per-partition DMA segments
    xT = nc.dram_tensor("xT", [128, NJ, KC, 512], bf16, kind="ExternalInput")
    wq = nc.dram_tensor("wq", [128, HPC, KC, 128], bf16, kind="ExternalInput")
    wr = nc.dram_tensor("wr", [128, HPC, KC, 128], bf16, kind="ExternalInput")
    cosr = nc.dram_tensor("cosr", [128, N], bf16, kind="ExternalInput")
    sinr = nc.dram_tensor("sinr", [128, N], bf16, kind="ExternalInput")
    maskt = nc.dram_tensor("maskt", [128, NJ, 8], bf16, kind="ExternalInput")
    out = nc.dram_tensor("out", [HPC, N, N], bf16, kind="ExternalOutput")
    # bounce rows: per-head 1/|q_n| and 1/|k_n| for partition-broadcast reads
    rqd = nc.dram_tensor("rqd", [HPC, N], bf16)
    rkd = nc.dram_tensor("rkd", [HPC, N], bf16)

    with tile.TileContext(nc) as tc:
        with (
            tc.tile_pool(name="singles", bufs=1) as singles,
            tc.tile_pool(name="persist", bufs=2) as persist,
        ):
            wqt = singles.tile([128, HPC, KC, 128], bf16)
            nc.sync.dma_start(out=wqt[:], in_=wq[:])
            wrt = singles.tile([128, HPC, KC, 128], bf16)
            nc.sync.dma_start(out=wrt[:], in_=wr[:])
            mask4 = singles.tile([128, NJ, 8], bf16)
            nc.sync.dma_start(out=mask4[:], in_=maskt[:])
            cost = singles.tile([128, N], bf16)
            sint = singles.tile([128, N], bf16)
            xt = singles.tile([128, NJ, KC, 512], bf16)
            for j in range(NJ):
                nc.sync.dma_start(out=xt[:, j, :, :], in_=xT[:, j, :, :])
                js = slice(j * 512, (j + 1) * 512)
                nc.sync.dma_start(out=cost[:, js], in_=cosr[:, js])
                nc.sync.dma_start(out=sint[:, js], in_=sinr[:, js])

            # per-head persistent tiles: qr = [q-dims | k-dims] x n (bf16,
            # q rows normalized in place), kt = normalized k at partitions
            # 0-63.
            qr_t, kt_t = {}, {}

            # ---------------- prologue: both heads' chains ----------------
            with (
                tc.tile_pool(name="chain", bufs=2) as chain_pool,
                tc.tile_pool(name="pq", bufs=2, space="PSUM") as pq_pool,
                tc.tile_pool(name="pr", bufs=2, space="PSUM") as pr_pool,
                tc.tile_pool(name="pnsq", bufs=2, space="PSUM") as pnsq_pool,
            ):
                for t in range(HPC):
                    qr = persist.tile([128, N], bf16, tag="qr", name=f"qr{t}")
                    kt = persist.tile([64, N], bf16, tag="kt", name=f"kt{t}")
                    qr_t[t], kt_t[t] = qr, kt
                    nsq_ps = pnsq_pool.tile([8, 512], f32, tag="nsq")
                    for j in range(NJ):
                        js = slice(j * 512, (j + 1) * 512)
                        qk_ps = pq_pool.tile([128, 512], f32, tag="pq")
                        rot_ps = pr_pool.tile([128, 512], f32, tag="pr")
                        for k in range(KC):
                            nc.tensor.matmul(
                                qk_ps[:], lhsT=wqt[:, t, k, :], rhs=xt[:, j, k, :],
                                start=(k == 0), stop=(k == KC - 1),
                            )
                        for k in range(KC):
                            nc.tensor.matmul(
                                rot_ps[:], lhsT=wrt[:, t, k, :], rhs=xt[:, j, k, :],
                                start=(k == 0), stop=(k == KC - 1),
                            )
                        # rotary: qr = qk*cos + rot(qk)*sin
                        t1 = chain_pool.tile([128, 512], bf16, tag="t1")
                        nc.vector.tensor_mul(t1[:], qk_ps[:], cost[:, js])
                        t2 = chain_pool.tile([128, 512], bf16, tag="t2")
                        nc.vector.tensor_mul(t2[:], rot_ps[:], sint[:, js])
                        nc.gpsimd.tensor_add(qr[:, js], t1[:], t2[:])
                        # squared entries (scalar engine; square shares the
                        # ln/exp ACT table set)
                        sq = chain_pool.tile([128, 512], bf16, tag="sq")
                        nc.scalar.activation(out=sq[:], in_=qr[:, js], func=AF.Square)
                        # accumulate |q|^2, |k|^2 per chunk into rows 2j,2j+1
                        nc.tensor.matmul(
                            nsq_ps[:], lhsT=mask4[:, j, :], rhs=sq[:],
                            start=(j == 0), stop=(j == NJ - 1),
                        )
                    # rinorm = nsq^-0.5 = exp(-0.5*ln(nsq)) for all chunks
                    # and both q/k at once
                    lnn = chain_pool.tile([8, 512], f32, tag="lnn")
                    nc.scalar.activation(out=lnn[:], in_=nsq_ps[:], func=AF.Ln)
                    rin = chain_pool.tile([8, 512], bf16, tag="rin")
                    nc.scalar.activation(out=rin[:], in_=lnn[:], func=AF.Exp, scale=-0.5)
                    for j in range(NJ):
                        js = slice(j * 512, (j + 1) * 512)
                        nc.sync.dma_start(out=rqd[t, js], in_=rin[2 * j:2 * j + 1, :])
                        nc.sync.dma_start(out=rkd[t, js], in_=rin[2 * j + 1:2 * j + 2, :])
                    for j in range(NJ):
                        js = slice(j * 512, (j + 1) * 512)
                        # q-normalize in place (exp scale is then 1/64 const)
                        bq = chain_pool.tile([64, 512], bf16, tag="bq")
                        nc.sync.dma_start(
                            out=bq[:], in_=rqd[t:t + 1, js].to_broadcast([64, 512])
                        )
                        nc.vector.tensor_mul(qr[0:64, js], qr[0:64, js], bq[:])
                        # k: shift to partitions 0-63 and normalize
                        ks = chain_pool.tile([64, 512], bf16, tag="ks")
                        nc.sync.dma_start(out=ks[:], in_=qr[64:128, js])
                        bk = chain_pool.tile([64, 512], bf16, tag="bk")
                        nc.sync.dma_start(
                            out=bk[:], in_=rkd[t:t + 1, js].to_broadcast([64, 512])
                        )
                        nc.vector.tensor_mul(kt[:, js], ks[:], bk[:])

            # ---------------- score phase: pure exp pipeline ----------------
            with (
                tc.tile_pool(name="exp", bufs=3) as exp_pool,
                tc.tile_pool(name="outp", bufs=4) as out_pool,
                tc.tile_pool(name="small", bufs=4) as small,
                tc.tile_pool(name="psc", bufs=2, space="PSUM") as psc,
            ):
                for t in range(HPC):
                    qr, kt = qr_t[t], kt_t[t]
                    for i in range(NT):
                        isl = slice(i * 128, (i + 1) * 128)
                        sc_ps = psc.tile([128, 2048], f32, tag="sc")
                        for j2 in range(4):
                            nc.tensor.matmul(
                                sc_ps[:, j2 * 512:(j2 + 1) * 512],
                                lhsT=qr[0:64, isl],
                                rhs=kt[:, j2 * 512:(j2 + 1) * 512],
                                start=True, stop=True,
                            )
                        et = exp_pool.tile([128, 2048], bf16, tag="et")
                        sums = small.tile([128, 1], f32, tag="sums")
                        nc.scalar.activation(
                            out=et[:], in_=sc_ps[:], func=AF.Exp,
                            scale=1.0 / D, accum_out=sums[:],
                        )
                        rs = small.tile([128, 1], f32, tag="rs")
                        nc.vector.reciprocal(out=rs[:], in_=sums[:])
                        ot = out_pool.tile([128, 2048], bf16, tag="ot")
                        nc.vector.tensor_scalar_mul(ot[:], et[:], rs[:])
                        nc.sync.dma_start(out=out[t, isl, :], in_=ot[:])

    nc.compile()
    return nc


def _get_nc():
    if "nc" not in _CACHE:
        _CACHE["nc"] = _build_nc()
    return _CACHE["nc"]


def _prep_inputs(x, rotary_cos, rotary_sin, W_qk):
    bf16 = ml_dtypes.bfloat16
    x = np.asarray(x, dtype=np.float32)
    cos = np.asarray(rotary_cos, dtype=np.float32)
    sin = np.asarray(rotary_sin, dtype=np.float32)
    W = np.asarray(W_qk, dtype=np.float32)

    cosr = np.concatenate([cos.T, cos.T], axis=0).astype(bf16)  # [128, N]
    sinr = np.concatenate([sin.T, sin.T], axis=0).astype(bf16)
    # nsq masks: variant j sums q-dims (partitions 0-63) into row 2j and
    # k-dims (partitions 64-127) into row 2j+1
    maskt = np.zeros((128, NJ, 8), dtype=bf16)
    for j in range(NJ):
        maskt[0:64, j, 2 * j] = 1.0
        maskt[64:128, j, 2 * j + 1] = 1.0

    # per-head weight lhsT chunks (and rotate_half-permuted variant),
    # stored partition-major: [p, head, kc, m]
    wq_h = np.empty((H, KC, 128, 128), dtype=np.float32)
    wr_h = np.empty((H, KC, 128, 128), dtype=np.float32)
    for h in range(H):
        wcat = np.concatenate(
            [W[h * D:(h + 1) * D], W[C + h * D:C + (h + 1) * D]], axis=0
        )  # [128, 512]
        wrot = np.empty_like(wcat)
        wrot[0:32] = -wcat[32:64]
        wrot[32:64] = wcat[0:32]
        wrot[64:96] = -wcat[96:128]
        wrot[96:128] = wcat[64:96]
        wq_h[h] = wcat.T.reshape(KC, 128, 128)
        wr_h[h] = wrot.T.reshape(KC, 128, 128)

    # xT partition-major chunked: [p, j, kc, nn]
    xTb = []
    for b in range(B):
        xT = x[b].T  # [C, N]
        xTb.append(np.ascontiguousarray(
            xT.reshape(KC, 128, NJ, 512).transpose(1, 2, 0, 3)
        ).astype(bf16))

    in_maps = []
    for core in range(NCORES):
        b = core // 4
        h0 = (core % 4) * HPC
        wqc = np.ascontiguousarray(
            wq_h[h0:h0 + HPC].transpose(2, 0, 1, 3)
        ).astype(bf16)  # [128, HPC, KC, 128]
        wrc = np.ascontiguousarray(
            wr_h[h0:h0 + HPC].transpose(2, 0, 1, 3)
        ).astype(bf16)
        in_maps.append({
            "xT": xTb[b],
            "wq": wqc,
            "wr": wrc,
            "cosr": cosr,
            "sinr": sinr,
            "maskt": maskt,
        })
    return in_maps


def run(x, rotary_cos, rotary_sin, W_qk, trace=False):
    from concourse.bass_utils import run_bass_kernel_spmd

    nc = _get_nc()
    in_maps = _prep_inputs(x, rotary_cos, rotary_sin, W_qk)
    res = run_bass_kernel_spmd(nc, in_maps, list(range(NCORES)), trace=trace)
    full = np.empty((B, H, N, N), dtype=np.float32)
    for core in range(NCORES):
        b = core // 4
        h0 = (core % 4) * HPC
        for t in range(HPC):
            full[b, h0 + t] = res.results[core]["out"][t].astype(np.float32)
    return full, res


def kernel(x, rotary_cos, rotary_sin, W_qk):
    full, _ = run(x, rotary_cos, rotary_sin, W_qk, trace=False)
    return full


# revision 8
# speedup vs baseline: 1.3590x; 1.0491x over previous
"""Fused rotary QK-projection + normalized dot-product attention softmax.

Computes softmax((q_hat @ k_hat^T) / 64) for q,k = L2-normalized rotary
projections of x, sharded over 8 NeuronCores as (batch x head-pair):
core c -> batch c//4, heads (2*(c%4), 2*(c%4)+1). No cross-core comms.

Structure: prologue computes both heads' projection chains (rotary,
norms, normalized k in SBUF); the score phase then runs pure
[128,2048] matmul->Exp->scale->DMA tiles, bottlenecked by the scalar
engine's exp throughput. 1/|q| folds into the per-row exp scale.
Output is written bf16 (halves DMA-out bytes) and upcast on host.

Self-contained: hardcodes shapes b=2, n=2048, dim=512, h=8, d=64.
"""

import numpy as np
import ml_dtypes

B = 2
N = 2048
C = 512           # model dim (contraction for projection)
H = 8             # heads
D = 64            # head dim
HPC = 2           # heads per core
NCORES = 8
KC = C // 128     # 4 contraction chunks of 128
NJ = N // 512     # 4 chain chunks of 512
NT = N // 128     # 16 q row-tiles

_CACHE = {}


def _setup_act_tables():
    """Point walrus at an act_info.json tweaked so Ln, Exp and Square all
    resolve to natural_log_exp_and_others (one shared ACT table set -> no
    ~2.7us table reloads between activation funcs). Set order/indices are
    kept identical; only the per-func set choice is steered."""
    import os
    import json
    import tempfile
    from pathlib import Path

    if os.environ.get("BASS_ACT_ROOT_JSON_PATH"):
        return
    from neuronxcc.driver.Job import Job

    src_dir = Path(Job.getPackageDir()) / "pwp" / "pwp_bin_trainium"
    src_json = src_dir / "act_info.json"
    if not src_json.exists():
        return
    info = json.loads(src_json.read_text())
    sets = info.get("act_func_sets", [])
    names = [s.get("name") for s in sets]
    if "natural_log_exp_and_others" not in names:
        return
    for s in sets:
        if s.get("name") != "natural_log_exp_and_others":
            s.get("act", {}).pop("exp", None)
            s.get("act", {}).pop("ln", None)
            s.get("act", {}).pop("square", None)
    dst_dir = Path(tempfile.mkdtemp(prefix="pwp_act_"))
    for f in src_dir.iterdir():
        if f.name != "act_info.json":
            (dst_dir / f.name).symlink_to(f)
    (dst_dir / "act_info.json").write_text(json.dumps(info))
    os.environ["BASS_ACT_ROOT_JSON_PATH"] = str(dst_dir / "act_info.json")


def _build_nc():
    import concourse.mybir as mybir
    import concourse.tile as tile
    from concourse import bacc

    _setup_act_tables()

    dt = mybir.dt
    f32, bf16 = dt.float32, dt.bfloat16
    AF = mybir.ActivationFunctionType

    nc = bacc.Bacc(None)
    # partition-major host layouts -> contiguous per-partition DMA segments
    xT = nc.dram_tensor("xT", [128, NJ, KC, 512], bf16, kind="ExternalInput")
    wq = nc.dram_tensor("wq", [128, HPC, KC, 128], bf16, kind="ExternalInput")
    wr = nc.dram_tensor("wr", [128, HPC, KC, 128], bf16, kind="ExternalInput")
    cosr = nc.dram_tensor("cosr", [128, N], bf16, kind="ExternalInput")
    sinr = nc.dram_tensor("sinr", [128, N], bf16, kind="ExternalInput")
    maskt = nc.dram_tensor("maskt", [128, NJ, 8], bf16, kind="ExternalInput")
    out = nc.dram_tensor("out", [HPC, N, N], bf16, kind="ExternalOutput")
    # bounce rows: per-head 1/|q_n| (gathered back as per-row-tile exp
    # scales) and 1/|k_n| (broadcast-read for the k-normalize)
    rqd = nc.dram_tensor("rqd", [HPC, N], bf16)
    rkd = nc.dram_tensor("rkd", [HPC, N], bf16)

    with tile.TileContext(nc) as tc:
        with (
            tc.tile_pool(name="singles", bufs=1) as singles,
            tc.tile_pool(name="persist", bufs=2) as persist,
        ):
            wqt = singles.tile([128, HPC, KC, 128], bf16)
            nc.sync.dma_start(out=wqt[:], in_=wq[:])
            wrt = singles.tile([128, HPC, KC, 128], bf16)
            nc.sync.dma_start(out=wrt[:], in_=wr[:])
            mask4 = singles.tile([128, NJ, 8], bf16)
            nc.sync.dma_start(out=mask4[:], in_=maskt[:])
            cost = singles.tile([128, N], bf16)
            sint = singles.tile([128, N], bf16)
            xt = singles.tile([128, NJ, KC, 512], bf16)
            for j in range(NJ):
                nc.sync.dma_start(out=xt[:, j, :, :], in_=xT[:, j, :, :])
                js = slice(j * 512, (j + 1) * 512)
                nc.sync.dma_start(out=cost[:, js], in_=cosr[:, js])
                nc.sync.dma_start(out=sint[:, js], in_=sinr[:, js])

            # per-head persistent tiles: qr = [q-dims | k-dims] x n (bf16),
            # kt = normalized k at partitions 0-63, rqs = per-row exp scales.
            qr_t, kt_t, rqs_t = {}, {}, {}

            # ---------------- prologue: both heads' chains ----------------
            with (
                tc.tile_pool(name="chain", bufs=2) as chain_pool,
                tc.tile_pool(name="pq", bufs=3, space="PSUM") as pq_pool,
                tc.tile_pool(name="pr", bufs=3, space="PSUM") as pr_pool,
                tc.tile_pool(name="pnsq", bufs=2, space="PSUM") as pnsq_pool,
            ):
                for t in range(HPC):
                    qr = persist.tile([128, N], bf16, tag="qr", name=f"qr{t}")
                    kt = persist.tile([64, N], bf16, tag="kt", name=f"kt{t}")
                    qr_t[t], kt_t[t] = qr, kt
                    nsq_ps = pnsq_pool.tile([8, 512], f32, tag="nsq")
                    ks_j = []
                    for j in range(NJ):
                        js = slice(j * 512, (j + 1) * 512)
                        qk_ps = pq_pool.tile([128, 512], f32, tag="pq")
                        rot_ps = pr_pool.tile([128, 512], f32, tag="pr")
                        for k in range(KC):
                            nc.tensor.matmul(
                                qk_ps[:], lhsT=wqt[:, t, k, :], rhs=xt[:, j, k, :],
                                start=(k == 0), stop=(k == KC - 1),
                            )
                        for k in range(KC):
                            nc.tensor.matmul(
                                rot_ps[:], lhsT=wrt[:, t, k, :], rhs=xt[:, j, k, :],
                                start=(k == 0), stop=(k == KC - 1),
                            )
                        # rotary: qr = qk*cos + rot(qk)*sin
                        t1 = chain_pool.tile([128, 512], bf16, tag="t1")
                        nc.vector.tensor_mul(t1[:], qk_ps[:], cost[:, js])
                        t2 = chain_pool.tile([128, 512], bf16, tag="t2")
                        nc.vector.tensor_mul(t2[:], rot_ps[:], sint[:, js])
                        nc.gpsimd.tensor_add(qr[:, js], t1[:], t2[:])
                        # k-half shifted to partitions 0-63 early (tensor
                        # engine queue; overlapped with later chunks)
                        ks = chain_pool.tile([64, 512], bf16, tag="ks", bufs=4)
                        nc.sync.dma_start(out=ks[:], in_=qr[64:128, js])
                        ks_j.append(ks)
                        # squared entries (scalar engine; square shares the
                        # ln/exp ACT table set)
                        sq = chain_pool.tile([128, 512], bf16, tag="sq")
                        nc.scalar.activation(out=sq[:], in_=qr[:, js], func=AF.Square)
                        # accumulate |q|^2, |k|^2 per chunk into rows 2j,2j+1
                        nc.tensor.matmul(
                            nsq_ps[:], lhsT=mask4[:, j, :], rhs=sq[:],
                            start=(j == 0), stop=(j == NJ - 1),
                        )
                    # rinorm = nsq^-0.5 = exp(-0.5*ln(nsq)) for all chunks
                    # and both q/k at once
                    lnn = chain_pool.tile([8, 512], f32, tag="lnn")
                    nc.scalar.activation(out=lnn[:], in_=nsq_ps[:], func=AF.Ln)
                    rin = chain_pool.tile([8, 512], bf16, tag="rin")
                    nc.scalar.activation(out=rin[:], in_=lnn[:], func=AF.Exp, scale=-0.5)
                    for j in range(NJ):
                        js = slice(j * 512, (j + 1) * 512)
                        # 1/|q| row out for the per-row-tile gather (same
                        # sync queue as the gather read below: FIFO-ordered)
                        nc.sync.dma_start(out=rqd[t, js], in_=rin[2 * j:2 * j + 1, :])
                        # k-normalize: bounce 1/|k| through DRAM and
                        # broadcast-read, on the scalar-engine DMA queue
                        # (FIFO-ordered there, off the busy sync queue)
                        nc.scalar.dma_start(out=rkd[t, js], in_=rin[2 * j + 1:2 * j + 2, :])
                        bk = chain_pool.tile([64, 512], bf16, tag="bk")
                        nc.scalar.dma_start(
                            out=bk[:], in_=rkd[t:t + 1, js].to_broadcast([64, 512])
                        )
                        nc.vector.tensor_mul(kt[:, js], ks_j[j][:], bk[:])
                    # per-row-tile exp scales: gather 1/|q| across partitions
                    rqt = chain_pool.tile([128, NT], bf16, tag="rqt")
                    nc.sync.dma_start(
                        out=rqt[:], in_=rqd[t].rearrange("(i p) -> p i", p=128)
                    )
                    rqs = persist.tile([128, NT], f32, tag="rqs", name=f"rqs{t}")
                    nc.vector.tensor_scalar_mul(rqs[:], rqt[:], 1.0 / D)
                    rqs_t[t] = rqs

            # ---------------- score phase: pure exp pipeline ----------------
            with (
                tc.tile_pool(name="exp", bufs=3) as exp_pool,
                tc.tile_pool(name="outp", bufs=4) as out_pool,
                tc.tile_pool(name="small", bufs=4) as small,
                tc.tile_pool(name="psc", bufs=2, space="PSUM") as psc,
            ):
                for t in range(HPC):
                    qr, kt, rqs = qr_t[t], kt_t[t], rqs_t[t]
                    for i in range(NT):
                        isl = slice(i * 128, (i + 1) * 128)
                        sc_ps = psc.tile([128, 2048], f32, tag="sc")
                        for j2 in range(4):
                            nc.tensor.matmul(
                                sc_ps[:, j2 * 512:(j2 + 1) * 512],
                                lhsT=qr[0:64, isl],
                                rhs=kt[:, j2 * 512:(j2 + 1) * 512],
                                start=True, stop=True,
                            )
                        et = exp_pool.tile([128, 2048], bf16, tag="et")
                        sums = small.tile([128, 1], f32, tag="sums")
                        nc.scalar.activation(
                            out=et[:], in_=sc_ps[:], func=AF.Exp,
                            scale=rqs[:, i:i + 1], accum_out=sums[:],
                        )
                        rs = small.tile([128, 1], f32, tag="rs")
                        nc.vector.reciprocal(out=rs[:], in_=sums[:])
                        ot = out_pool.tile([128, 2048], bf16, tag="ot")
                        nc.vector.tensor_scalar_mul(ot[:], et[:], rs[:])
                        nc.sync.dma_start(out=out[t, isl, :], in_=ot[:])

    nc.compile()
    return nc


def _get_nc():
    if "nc" not in _CACHE:
        _CACHE["nc"] = _build_nc()
    return _CACHE["nc"]


def _prep_inputs(x, rotary_cos, rotary_sin, W_qk):
    bf16 = ml_dtypes.bfloat16
    x = np.asarray(x, dtype=np.float32)
    cos = np.asarray(rotary_cos, dtype=np.float32)
    sin = np.asarray(rotary_sin, dtype=np.float32)
    W = np.asarray(W_qk, dtype=np.float32)

    cosr = np.concatenate([cos.T, cos.T], axis=0).astype(bf16)  # [128, N]
    sinr = np.concatenate([sin.T, sin.T], axis=0).astype(bf16)
    # nsq masks: variant j sums q-dims (partitions 0-63) into row 2j and
    # k-dims (partitions 64-127) into row 2j+1
    maskt = np.zeros((128, NJ, 8), dtype=bf16)
    for j in range(NJ):
        maskt[0:64, j, 2 * j] = 1.0
        maskt[64:128, j, 2 * j + 1] = 1.0

    # per-head weight lhsT chunks (and rotate_half-permuted variant),
    # stored partition-major: [p, head, kc, m]
    wq_h = np.empty((H, KC, 128, 128), dtype=np.float32)
    wr_h = np.empty((H, KC, 128, 128), dtype=np.float32)
    for h in range(H):
        wcat = np.concatenate(
            [W[h * D:(h + 1) * D], W[C + h * D:C + (h + 1) * D]], axis=0
        )  # [128, 512]
        wrot = np.empty_like(wcat)
        wrot[0:32] = -wcat[32:64]
        wrot[32:64] = wcat[0:32]
        wrot[64:96] = -wcat[96:128]
        wrot[96:128] = wcat[64:96]
        wq_h[h] = wcat.T.reshape(KC, 128, 128)
        wr_h[h] = wrot.T.reshape(KC, 128, 128)

    # xT partition-major chunked: [p, j, kc, nn]
    xTb = []
    for b in range(B):
        xT = x[b].T  # [C, N]
        xTb.append(np.ascontiguousarray(
            xT.reshape(KC, 128, NJ, 512).transpose(1, 2, 0, 3)
        ).astype(bf16))

    in_maps = []
    for core in range(NCORES):
        b = core // 4
        h0 = (core % 4) * HPC
        wqc = np.ascontiguousarray(
            wq_h[h0:h0 + HPC].transpose(2, 0, 1, 3)
        ).astype(bf16)  # [128, HPC, KC, 128]
        wrc = np.ascontiguousarray(
            wr_h[h0:h0 + HPC].transpose(2, 0, 1, 3)
        ).astype(bf16)
        in_maps.append({
            "xT": xTb[b],
            "wq": wqc,
            "wr": wrc,
            "cosr": cosr,
            "sinr": sinr,
            "maskt": maskt,
        })
    return in_maps


def run(x, rotary_cos, rotary_sin, W_qk, trace=False):
    from concourse.bass_utils import run_bass_kernel_spmd

    nc = _get_nc()
    in_maps = _prep_inputs(x, rotary_cos, rotary_sin, W_qk)
    res = run_bass_kernel_spmd(nc, in_maps, list(range(NCORES)), trace=trace)
    full = np.empty((B, H, N, N), dtype=np.float32)
    for core in range(NCORES):
        b = core // 4
        h0 = (core % 4) * HPC
        for t in range(HPC):
            full[b, h0 + t] = res.results[core]["out"][t].astype(np.float32)
    return full, res


def kernel(x, rotary_cos, rotary_sin, W_qk):
    full, _ = run(x, rotary_cos, rotary_sin, W_qk, trace=False)
    return full
